# revision 1
# baseline (speedup 1.0000x reference)
"""Trainium2 Bass kernel for CRF negative-log-likelihood loss.

Problem: nn_CRF (B=512, L=1024, T=48), data-parallel over 8 NeuronCores
(64 batch rows per core). Each core computes a scalar partial loss; the
host sums the 8 partials.

Per-core algorithm (validated against a float64 numpy reference):
  forward (partition function):
    exp-domain scan A_t[j,b] = sum_i E[i,j] A_{t-1}[i,b] * F_t[j,b]
    with E = exp(trans - log T) as stationary PE weights extended with an
    exp(end) capture column and a ones colsum column; F_t = exp(feat_t - MU)
    produced by bulk PE transposes + fused ACT exp-copies. Per-b
    renormalization every R steps is folded into the F tile DELTA steps
    later (off the critical path); log-scales accumulate via the
    suffix-mask identity sum_t ind[t,b]*logS(t)[b] =
    sum_rho log s_rho[b] * maskT[apply_rho][b]. The mask never enters the
    scan: terminal alphas are recovered by indicator-selection
    (ind = maskT[t] - maskT[t+1]) over captured end-rows.
  gold (numerator): one-hot tiles via per-partition tag scalars
    (tensor_scalar is_equal), a bigram-count matmul C = OHu^T @ OHm_shift
    accumulated in PSUM then contracted with trans, and fused
    tensor_tensor_reduce feat gathers. Everything reduces through
    ones-matmuls into PSUM scalar accumulators.
"""

import math

import numpy as np

import concourse.bacc as bacc
import concourse.mybir as mybir
import concourse.tile as tile
from concourse.bass_utils import run_bass_kernel_spmd

F32 = mybir.dt.float32
I32 = mybir.dt.int32
AF = mybir.ActivationFunctionType
OP = mybir.AluOpType

B_FULL = 512
N_CORES = 8
BC = B_FULL // N_CORES  # 64
L_FULL = 1024
T = 48

MU = 0.51                # per-step feat shift folded into F (calibrated
                         # so mean per-step log-gain ~ 0: keeps Ln inputs
                         # inside the ACT spline accurate range)
A_SHIFT = math.log(T)    # shift folded into E
R = 16                   # renorm period (steps)
DELTA = 8                # renorm application delay (steps)
FCHUNK = 32              # timesteps per F-prep DMA chunk


def build_program(L=L_FULL, Bc=BC, G=1, dbg=False):
    """Emit the full per-core program; returns the compiled Bacc object."""
    assert L % 128 == 0 and L % FCHUNK == 0
    Nb = Bc // G
    CAP0 = L // 2          # captures kept for t >= CAP0-1 (lengths >= L/2)
    assert CAP0 % 128 == 0
    n_tt = L // 128
    n_cap = (L - CAP0) // 128
    nchunks = L // FCHUNK

    nc = bacc.Bacc("TRN2", target_bir_lowering=False, debug=False)

    feats_d = nc.dram_tensor("feats", (Bc, L, T), F32, kind="ExternalInput")
    trans_d = nc.dram_tensor("trans", (T, T), F32, kind="ExternalInput")
    start_d = nc.dram_tensor("start", (T,), F32, kind="ExternalInput")
    end_d = nc.dram_tensor("end", (T,), F32, kind="ExternalInput")
    tags_d = nc.dram_tensor("tags", (Bc, L), I32, kind="ExternalInput")
    mask_d = nc.dram_tensor("mask", (Bc, L), I32, kind="ExternalInput")
    out_d = nc.dram_tensor("out", (1, 1), F32, kind="ExternalOutput")
    dbg_d = (nc.dram_tensor("dbg", (6, Bc), F32, kind="ExternalOutput")
             if dbg else None)

    feats_flat = feats_d.ap().rearrange("b l t -> b (l t)")

    # renorm schedule: at MM step t (t % R == 0, t+DELTA-1 < L) the colsum of
    # A_{t-1} is available; its reciprocal is folded into F at t-1+DELTA.
    renorm_ts = [t for t in range(R, L + 1, R) if t + DELTA - 1 < L]

    with tile.TileContext(nc) as tc:
        with (
            tc.tile_pool(name="const", bufs=1) as cp,
            tc.tile_pool(name="cpsum", bufs=1, space="PSUM") as cpp,
        ):
            # ---------------- constants ----------------
            iota48i = cp.tile((128, T), I32)
            nc.gpsimd.iota(iota48i[:, :], [[1, T]], channel_multiplier=0)
            iota48f = cp.tile((128, T), F32)
            nc.vector.tensor_copy(iota48f[:, :], iota48i[:, :])

            iotaLi = cp.tile((Bc, L), I32)
            nc.gpsimd.iota(iotaLi[:, :], [[1, L]], channel_multiplier=0)
            iotaLf = cp.tile((Bc, L), F32)
            nc.vector.tensor_copy(iotaLf[:, :], iotaLi[:, :])

            iota64i = cp.tile((64, 64), I32)
            nc.gpsimd.iota(iota64i[:, :], [[1, 64]], channel_multiplier=0)
            iotaPi = cp.tile((64, 1), I32)
            nc.gpsimd.iota(iotaPi[:, :], [[1, 1]], channel_multiplier=1)
            iota64f = cp.tile((64, 64), F32)
            nc.vector.tensor_copy(iota64f[:, :], iota64i[:, :])
            iotaPf = cp.tile((64, 1), F32)
            nc.vector.tensor_copy(iotaPf[:, :], iotaPi[:, :])
            identM = cp.tile((64, 64), F32)
            nc.vector.tensor_scalar(
                identM[:, :], iota64f[:, :], iotaPf[:, :], None, OP.is_equal)

            ones128 = cp.tile((128, 1), F32)
            nc.vector.memset(ones128[:, :], 1.0)
            onesrow = cp.tile((1, T), F32)
            nc.vector.memset(onesrow[:, :], 1.0)

            # activation bias tiles (arbitrary float biases need APs)
            bias_a = cp.tile((T, 1), F32)
            nc.vector.memset(bias_a[:, :], -A_SHIFT)
            bias_mu = cp.tile((T, 1), F32)
            nc.vector.memset(bias_mu[:, :], -MU)

            # ---------------- params ----------------
            trans_sb = cp.tile((T, T), F32)
            nc.sync.dma_start(trans_sb[:, :], trans_d.ap())
            e_mat = cp.tile((T, T), F32)
            nc.scalar.activation(e_mat[:, :], trans_sb[:, :], AF.Exp,
                                 bias=bias_a[:, :])
            end_sb = cp.tile((T, 1), F32)
            nc.sync.dma_start(end_sb[:, :], end_d.ap().unsqueeze(1))
            expend = cp.tile((T, 1), F32)
            nc.scalar.activation(expend[:, :], end_sb[:, :], AF.Exp)
            ones48c = cp.tile((T, 1), F32)
            nc.vector.memset(ones48c[:, :], 1.0)

            start_sb = cp.tile((T, 1), F32)
            nc.sync.dma_start(start_sb[:, :],
                              start_d.ap().unsqueeze(1))
            expstart = cp.tile((T, 1), F32)
            nc.scalar.activation(expstart[:, :], start_sb[:, :], AF.Exp)

            startbc = cp.tile((Bc, T), F32)
            nc.sync.dma_start(
                startbc[:, :],
                start_d.ap().unsqueeze(0).partition_broadcast(Bc))
            endbc = cp.tile((Bc, T), F32)
            nc.sync.dma_start(
                endbc[:, :],
                end_d.ap().unsqueeze(0).partition_broadcast(Bc))

            # ---------------- tags / mask ----------------
            tags_i = cp.tile((Bc, L), I32)
            nc.sync.dma_start(tags_i[:, :], tags_d.ap())
            tagsf = cp.tile((Bc, L), F32)
            nc.vector.tensor_copy(tagsf[:, :], tags_i[:, :])
            mask_i = cp.tile((Bc, L), I32)
            nc.sync.dma_start(mask_i[:, :], mask_d.ap())
            maskf = cp.tile((Bc, L), F32)
            nc.vector.tensor_copy(maskf[:, :], mask_i[:, :])

            # transposed (128-timestep x Bc) tag/mask tiles
            prep_scope = tc.tile_pool(name="prepps", bufs=2, space="PSUM")
            ppp = prep_scope.__enter__()
            maskT = []
            tagsT = []
            for k in range(n_tt):
                ps = ppp.tile((128, Bc), F32, name=f"tp_ps_{k}", tag="tp_ps",
                              bufs=2)
                nc.tensor.transpose(ps[:, :], maskf[:, 128 * k:128 * (k + 1)],
                                    identM[:, :])
                mt = cp.tile((128, Bc), F32, name=f"maskT_{k}")
                nc.scalar.copy(mt[:, :], ps[:, :])
                maskT.append(mt)
                ps2 = ppp.tile((128, Bc), F32, name=f"tp_ps2_{k}",
                               tag="tp_ps", bufs=2)
                nc.tensor.transpose(ps2[:, :], tagsf[:, 128 * k:128 * (k + 1)],
                                    identM[:, :])
                tt = cp.tile((128, Bc), F32, name=f"tagsT_{k}")
                nc.scalar.copy(tt[:, :], ps2[:, :])
                tagsT.append(tt)

            # shifted (t+1) variants via partition-shift DMAs
            zero_row = cp.tile((1, Bc), F32)
            nc.vector.memset(zero_row[:, :], 0.0)
            maskTs = []
            tagsTs = []
            for k in range(n_tt):
                ms = cp.tile((128, Bc), F32, name=f"maskTs_{k}")
                nc.sync.dma_start(ms[0:127, :], maskT[k][1:128, :])
                ts_ = cp.tile((128, Bc), F32, name=f"tagsTs_{k}")
                nc.sync.dma_start(ts_[0:127, :], tagsT[k][1:128, :])
                if k + 1 < n_tt:
                    nc.sync.dma_start(ms[127:128, :], maskT[k + 1][0:1, :])
                    nc.sync.dma_start(ts_[127:128, :], tagsT[k + 1][0:1, :])
                else:
                    nc.sync.dma_start(ms[127:128, :], zero_row[:, :])
                    nc.sync.dma_start(ts_[127:128, :], zero_row[:, :])
                maskTs.append(ms)
                tagsTs.append(ts_)

            # masked tag tiles: tag + (1-mask)*100 makes the one-hot vanish
            tagsTm = []
            tagsTsm = []
            for k in range(n_tt):
                off = cp.tile((128, Bc), F32, name=f"moff_{k}")
                nc.vector.tensor_scalar(off[:, :], maskT[k][:, :], -100.0,
                                        100.0, OP.mult, OP.add)
                tm = cp.tile((128, Bc), F32, name=f"tagsTm_{k}")
                nc.vector.tensor_tensor(tm[:, :], tagsT[k][:, :], off[:, :],
                                        OP.add)
                tagsTm.append(tm)
                offs = cp.tile((128, Bc), F32, name=f"moffs_{k}")
                nc.vector.tensor_scalar(offs[:, :], maskTs[k][:, :], -100.0,
                                        100.0, OP.mult, OP.add)
                tms = cp.tile((128, Bc), F32, name=f"tagsTsm_{k}")
                nc.vector.tensor_tensor(tms[:, :], tagsTs[k][:, :],
                                        offs[:, :], OP.add)
                tagsTsm.append(tms)

            # indicator ind[t,b] = maskT[t] - maskT[t+1] (last row: maskT)
            ind = []
            for k in range(n_tt):
                it = cp.tile((128, Bc), F32, name=f"ind_{k}")
                nc.vector.tensor_tensor(it[:, :], maskT[k][:, :],
                                        maskTs[k][:, :], OP.subtract)
                ind.append(it)
            ind_c0 = cp.tile((1, Bc), F32)
            nc.sync.dma_start(ind_c0[:, :], ind[CAP0 // 128 - 1][127:128, :])

            # partition-0-aligned mask rows for each renorm fold time
            mrow = {}
            for t in renorm_ts:
                tf = t - 1 + DELTA
                mr = cp.tile((1, Bc), F32, name=f"mrow_{tf}")
                nc.sync.dma_start(mr[:, :],
                                  maskT[tf // 128][tf % 128:tf % 128 + 1, :])
                mrow[t] = mr

            # len row (1, Bc) via ones-matmul over maskT tiles
            len_ps = ppp.tile((1, Bc), F32, name="len_ps", tag="len_ps",
                              bufs=1)
            for k in range(n_tt):
                nc.tensor.matmul(len_ps[:, :], ones128[:, :], maskT[k][:, :],
                                 start=(k == 0), stop=(k == n_tt - 1),
                                 skip_group_check=True)
            lenm1_row = cp.tile((1, Bc), F32)
            nc.vector.tensor_scalar(lenm1_row[:, :], len_ps[:, :], 1.0, None,
                                    OP.subtract)
            prep_scope.__exit__(None, None, None)

            # persistent accumulators
            logsel = cp.tile((1, Bc), F32)
            nc.vector.memset(logsel[:, :], 0.0)
            feat_acc = cp.tile((128, Bc * n_tt), F32)
            misc_acc = cp.tile((Bc, 4), F32)

            c_ps = cpp.tile((T, T), F32, name="c_ps")  # bigram counts

            # =============== scan + F-prep + gold ===============
            # capture staging lives in DRAM: one row per captured step,
            # packed contiguously; split into (t, b) tiles in the end phase.
            ncap_steps = L - (CAP0 - 8)
            with tc.tile_pool(name="dramp", bufs=1, space="DRAM") as dp:
                cap_stage = dp.tile((1, ncap_steps * Bc), F32,
                                    name="cap_stage")
            with (
                tc.tile_pool(name="natp", bufs=3) as natp,
                tc.tile_pool(name="fpool", bufs=10) as fpool,
                tc.tile_pool(name="tpps", bufs=2, space="PSUM") as tpps,
                tc.tile_pool(name="scanps", bufs=1, space="PSUM") as scanps,
                tc.tile_pool(name="capps", bufs=2, space="PSUM") as capps,
                tc.tile_pool(name="rbcps", bufs=1, space="PSUM") as rbcps,
                tc.tile_pool(name="csps", bufs=1, space="PSUM") as csps,
                tc.tile_pool(name="apool", bufs=3) as apool,
                tc.tile_pool(name="fgp", bufs=6) as fgp,
                tc.tile_pool(name="ohp", bufs=8) as ohp,
                tc.tile_pool(name="scrp", bufs=2) as scrp,
            ):
                ftiles = {}

                def emit_fprep(c):
                    # one chunk = FCHUNK timesteps; F tiles hold 8 t each
                    natf = natp.tile((Bc, FCHUNK * T), F32, name="natf")
                    nc.sync.dma_start(
                        natf[:, :],
                        feats_flat[:, FCHUNK * T * c:FCHUNK * T * (c + 1)])
                    for q in range(FCHUNK // 8):
                        ps = tpps.tile((T, 512), F32, name="tp")
                        for k in range(8):
                            blk = q * 8 + k
                            nc.tensor.transpose(
                                ps[:, 64 * k:64 * k + Bc],
                                natf[:, T * blk:T * (blk + 1)],
                                identM[:, :])
                        ft = fpool.tile((T, 512), F32, name="ftile")
                        nc.scalar.activation(ft[:, :], ps[:, :], AF.Exp,
                                             bias=bias_mu[:, :])
                        ftiles[c * (FCHUNK // 8) + q] = ft

                def f_slice(t, g=0):
                    ft = ftiles[t // 8]
                    c0 = (t % 8) * 64
                    return ft[0:T, c0 + g * Nb:c0 + (g + 1) * Nb]

                emit_fprep(0)
                emit_fprep(1)

                # A0 = exp(start) * F_0
                a_prev = apool.tile((T, Bc), F32, name="a_t")
                nc.vector.tensor_scalar(
                    a_prev[:, :], ftiles[0][0:T, 0:Bc], expstart[:, :],
                    None, OP.mult)

                for t in range(1, L + 1):
                    if t % FCHUNK == 1:
                        c = (t - 1) // FCHUNK + 2
                        if c < nchunks:
                            emit_fprep(c)
                    tprev = t - 1
                    # end-capture of A_{t-1}: ring row in PSUM, flushed to
                    # SBUF staging by ACT once per 8 steps
                    if tprev >= CAP0 - 8:
                        slot = (tprev - (CAP0 - 8)) % 8
                        if slot == 0:
                            cap_ring = capps.tile((1, 8 * Bc), F32,
                                                  name="cap_ring")
                        nc.tensor.matmul(
                            cap_ring[0:1, slot * Bc:(slot + 1) * Bc],
                            expend[:, :], a_prev[:, :],
                            start=True, stop=True, skip_group_check=True)
                        if slot == 7:
                            blk = (tprev - (CAP0 - 8)) // 8
                            crow = scrp.tile((1, 8 * Bc), F32, name="crow",
                                             tag="crow")
                            nc.scalar.copy(crow[0:1, :], cap_ring[0:1, :])
                            nc.sync.dma_start(
                                cap_stage[0:1, blk * 8 * Bc:
                                          (blk + 1) * 8 * Bc],
                                crow[0:1, :])
                    # renorm: colsum of A_{t-1} via ones-matmul, fold at t-1+DELTA
                    if t in mrow:
                        tf = t - 1 + DELTA
                        for g in range(G):
                            gs = slice(g * Nb, (g + 1) * Nb)
                            cs = csps.tile((1, Nb), F32, name="cs")
                            nc.tensor.matmul(
                                cs[:, :], ones48c[:, :], a_prev[:, gs],
                                start=True, stop=True, skip_group_check=True)
                            r_sb = scrp.tile((1, Nb), F32, name="r_sb",
                                             tag="renorm")
                            nc.vector.reciprocal(r_sb[:, :], cs[:, :])
                            ls = scrp.tile((1, Nb), F32, name="ls",
                                           tag="renorm")
                            nc.scalar.activation(ls[:, :], cs[:, :], AF.Ln)
                            nc.vector.tensor_tensor(
                                ls[:, :], ls[:, :], mrow[t][:, gs], OP.mult)
                            nc.vector.tensor_tensor(
                                logsel[:, gs], logsel[:, gs], ls[:, :],
                                OP.add)
                            rbc = rbcps.tile((T, Nb), F32, name="rbc")
                            nc.tensor.matmul(
                                rbc[:, :], onesrow[:, :], r_sb[:, :],
                                start=True, stop=True, skip_group_check=True)
                            nc.vector.tensor_tensor(
                                f_slice(tf, g), f_slice(tf, g), rbc[:, :],
                                OP.mult)
                    if t < L:
                        a_cur = apool.tile((T, Bc), F32, name="a_t")
                        for g in range(G):
                            ps = scanps.tile((T, Nb), F32, name="mm_ps")
                            nc.tensor.matmul(
                                ps[:, :], e_mat[:, :],
                                a_prev[:, g * Nb:(g + 1) * Nb],
                                start=True, stop=True, skip_group_check=True)
                            nc.vector.tensor_tensor(
                                a_cur[:, g * Nb:(g + 1) * Nb],
                                ps[:, :], f_slice(t, g), OP.mult)
                        a_prev = a_cur

                # =============== gold path ===============
                nmm = 0
                for b in range(Bc):
                    for ck in range(n_tt):
                        fg = fgp.tile((128, T), F32, name="fg")
                        nc.sync.dma_start(
                            fg[:, :],
                            feats_flat[b:b + 1,
                                       128 * T * ck:128 * T * (ck + 1)]
                            .rearrange("o (p f) -> (o p) f", f=T))
                        tcol = tagsT[ck][:, b:b + 1]
                        ohu = ohp.tile((128, T), F32, name="ohu")
                        nc.vector.tensor_scalar(ohu[:, :], iota48f[:, :],
                                                tcol, None, OP.is_equal)
                        ohms = ohp.tile((128, T), F32, name="ohms")
                        nc.vector.tensor_scalar(
                            ohms[:, :], iota48f[:, :],
                            tagsTsm[ck][:, b:b + 1], None, OP.is_equal)
                        nc.tensor.matmul(c_ps[:, :], ohu[:, :], ohms[:, :],
                                         start=(nmm == 0), stop=False,
                                         skip_group_check=True)
                        nmm += 1
                        scr = scrp.tile((128, T), F32, name="scr", tag="scr")
                        nc.vector.scalar_tensor_tensor(
                            scr[:, :], iota48f[:, :],
                            tagsTm[ck][:, b:b + 1], fg[:, :],
                            OP.is_equal, OP.mult,
                            accum_out=feat_acc[:, b * n_tt + ck:
                                               b * n_tt + ck + 1])
                zrow = cp.tile((1, T), F32)
                nc.vector.memset(zrow[:, :], 0.0)
                nc.tensor.matmul(c_ps[:, :], zrow[:, :], zrow[:, :],
                                 start=False, stop=True,
                                 skip_group_check=True)

                # gold misc terms (b-partition layout)
                featlast = fgp.tile((Bc, T), F32, name="featlast")
                nc.sync.dma_start(featlast[:, :],
                                  feats_flat[:, (L - 1) * T:L * T])
                scrb = scrp.tile((Bc, T), F32, name="scrb", tag="scrb")
                nc.vector.scalar_tensor_tensor(
                    scrb[:, :], iota48f[0:Bc, :], tagsf[:, 0:1],
                    startbc[:, :], OP.is_equal, OP.mult,
                    accum_out=misc_acc[:, 0:1])
                scrb2 = scrp.tile((Bc, T), F32, name="scrb2", tag="scrb")
                mtagl = ohp.tile((Bc, 1), F32, name="mtagl")
                nc.vector.tensor_scalar(mtagl[:, :], maskf[:, L - 1:L],
                                        -100.0, 100.0, OP.mult, OP.add)
                nc.vector.tensor_tensor(mtagl[:, :], mtagl[:, :],
                                        tagsf[:, L - 1:L], OP.add)
                fcor = ohp.tile((Bc, 1), F32, name="fcor")
                nc.vector.scalar_tensor_tensor(
                    scrb2[:, :], iota48f[0:Bc, :], mtagl[:, :],
                    featlast[:, :], OP.is_equal, OP.mult,
                    accum_out=fcor[:, :])
                nc.vector.tensor_scalar(misc_acc[:, 3:4], fcor[:, :], -1.0,
                                        None, OP.mult)
                lenb = cp.tile((Bc, 1), F32)
                nc.vector.tensor_reduce(lenb[:, :], maskf[:, :],
                                        mybir.AxisListType.X, OP.add)
                lm1 = cp.tile((Bc, 1), F32)
                nc.vector.tensor_scalar(lm1[:, :], lenb[:, :], 1.0, None,
                                        OP.subtract)
                scrL = cp.tile((Bc, L), F32)
                lt = cp.tile((Bc, 1), F32)
                nc.vector.scalar_tensor_tensor(
                    scrL[:, :], iotaLf[:, :], lm1[:, :], tagsf[:, :],
                    OP.is_equal, OP.mult, accum_out=lt[:, :])
                scrb3 = scrp.tile((Bc, T), F32, name="scrb3", tag="scrb")
                nc.vector.scalar_tensor_tensor(
                    scrb3[:, :], iota48f[0:Bc, :], lt[:, :], endbc[:, :],
                    OP.is_equal, OP.mult, accum_out=misc_acc[:, 1:2])
                scrb4 = scrp.tile((Bc, T), F32, name="scrb4", tag="scrb")
                fe0 = cp.tile((Bc, 1), F32)
                nc.vector.scalar_tensor_tensor(
                    scrb4[:, :], iota48f[0:Bc, :], lt[:, :], featlast[:, :],
                    OP.is_equal, OP.mult, accum_out=fe0[:, :])
                nc.vector.tensor_tensor(misc_acc[:, 2:3], fe0[:, :],
                                        maskf[:, L - 1:L], OP.mult)

            # =============== end phase ===============
            with (
                tc.tile_pool(name="endp", bufs=2) as ep,
                tc.tile_pool(name="endps", bufs=1, space="PSUM") as epp,
            ):
                gold_ps = epp.tile((1, 1), F32, name="gold_ps")
                scrT = ep.tile((T, T), F32, name="scrT")
                cacc = ep.tile((T, 1), F32, name="cacc")
                nc.vector.tensor_tensor(scrT[:, :], c_ps[:, :],
                                        trans_sb[:, :], OP.mult)
                nc.vector.tensor_reduce(cacc[:, :], scrT[:, :],
                                        mybir.AxisListType.X, OP.add)
                nc.tensor.matmul(gold_ps[:, :], ones128[0:T, :], cacc[:, :],
                                 start=True, stop=False,
                                 skip_group_check=True)
                fred = ep.tile((128, 1), F32, name="fred")
                nc.vector.tensor_reduce(fred[:, :], feat_acc[:, :],
                                        mybir.AxisListType.X, OP.add)
                nc.tensor.matmul(gold_ps[:, :], ones128[:, :], fred[:, :],
                                 start=False, stop=False,
                                 skip_group_check=True)
                mred = ep.tile((Bc, 1), F32, name="mred")
                nc.vector.tensor_reduce(mred[:, :], misc_acc[:, :],
                                        mybir.AxisListType.X, OP.add)
                nc.tensor.matmul(gold_ps[:, :], ones128[0:Bc, :], mred[:, :],
                                 start=False, stop=True,
                                 skip_group_check=True)

                fwd_ps = epp.tile((1, Bc), F32, name="fwd_ps")
                for m in range(n_cap):
                    capt = ep.tile((128, Bc), F32, name="capt", tag="capt")
                    nc.sync.dma_start(
                        capt[:, :],
                        cap_stage[0:1, (8 + 128 * m) * Bc:
                                  (8 + 128 * (m + 1)) * Bc]
                        .rearrange("o (p f) -> o p f", f=Bc))
                    lc = ep.tile((128, Bc), F32, name="lc", tag="lc")
                    nc.scalar.activation(lc[:, :], capt[:, :], AF.Ln)
                    pr = ep.tile((128, Bc), F32, name="pr", tag="pr")
                    nc.vector.tensor_tensor(
                        pr[:, :], lc[:, :], ind[CAP0 // 128 + m][:, :],
                        OP.mult)
                    nc.tensor.matmul(fwd_ps[:, :], ones128[:, :], pr[:, :],
                                     start=(m == 0), stop=(m == n_cap - 1),
                                     skip_group_check=True)
                fwd_sel = ep.tile((1, Bc), F32, name="fwd_sel")
                nc.scalar.copy(fwd_sel[:, :], fwd_ps[:, :])
                lc0 = ep.tile((1, Bc), F32, name="lc0")
                cap0t = ep.tile((1, Bc), F32, name="cap0t")
                nc.sync.dma_start(cap0t[:, :], cap_stage[0:1, 7 * Bc:8 * Bc])
                nc.scalar.activation(lc0[:, :], cap0t[:, :], AF.Ln)
                nc.vector.tensor_tensor(lc0[:, :], lc0[:, :], ind_c0[:, :],
                                        OP.mult)
                nc.vector.tensor_tensor(fwd_sel[:, :], fwd_sel[:, :],
                                        lc0[:, :], OP.add)
                nc.vector.tensor_tensor(fwd_sel[:, :], fwd_sel[:, :],
                                        logsel[:, :], OP.add)
                shifts = ep.tile((1, Bc), F32, name="shifts")
                nc.vector.tensor_scalar(shifts[:, :], lenm1_row[:, :],
                                        A_SHIFT + MU, MU, OP.mult, OP.add)
                nc.vector.tensor_tensor(fwd_sel[:, :], fwd_sel[:, :],
                                        shifts[:, :], OP.add)
                fwd_tot = ep.tile((1, 1), F32, name="fwd_tot")
                nc.vector.tensor_reduce(fwd_tot[:, :], fwd_sel[:, :],
                                        mybir.AxisListType.X, OP.add)
                loss = ep.tile((1, 1), F32, name="loss")
                nc.vector.tensor_tensor(loss[:, :], fwd_tot[:, :],
                                        gold_ps[:, :], OP.subtract)
                nc.sync.dma_start(out_d.ap(), loss[:, :])
                if dbg:
                    gsb = ep.tile((1, 1), F32, name="gsb")
                    nc.scalar.copy(gsb[:, :], gold_ps[:, :])
                    fsel0 = ep.tile((1, Bc), F32, name="fsel0")
                    nc.scalar.copy(fsel0[:, :], fwd_ps[:, :])
                    nc.sync.dma_start(dbg_d.ap()[0:1, :], logsel[:, :])
                    nc.sync.dma_start(dbg_d.ap()[1:2, :], fwd_sel[:, :])
                    nc.sync.dma_start(dbg_d.ap()[2:3, :], lenm1_row[:, :])
                    nc.sync.dma_start(dbg_d.ap()[3:4, :], lc0[:, :])
                    nc.sync.dma_start(dbg_d.ap()[4:5, :], fsel0[:, :])
                    nc.sync.dma_start(dbg_d.ap()[5:6, 0:1], gsb[:, :])

    nc.compile()
    return nc


def shard_inputs(feats, transitions, start_transitions, end_transitions,
                 tags, mask, n_cores=N_CORES):
    feats = np.ascontiguousarray(np.asarray(feats, dtype=np.float32))
    transitions = np.ascontiguousarray(
        np.asarray(transitions, dtype=np.float32))
    start_transitions = np.ascontiguousarray(
        np.asarray(start_transitions, dtype=np.float32))
    end_transitions = np.ascontiguousarray(
        np.asarray(end_transitions, dtype=np.float32))
    tags = np.ascontiguousarray(np.asarray(tags).astype(np.int32))
    mask = np.ascontiguousarray(np.asarray(mask).astype(np.int32))
    Bc = feats.shape[0] // n_cores
    in_maps = []
    for c in range(n_cores):
        s = slice(c * Bc, (c + 1) * Bc)
        in_maps.append({
            "feats": feats[s],
            "trans": transitions,
            "start": start_transitions,
            "end": end_transitions,
            "tags": tags[s],
            "mask": mask[s],
        })
    return in_maps, feats.shape


def kernel(feats, transitions, start_transitions, end_transitions, tags,
           mask, **_ignored):
    in_maps, (Bf, L, _) = shard_inputs(
        feats, transitions, start_transitions, end_transitions, tags, mask)
    nc = build_program(L=L, Bc=Bf // N_CORES)
    res = run_bass_kernel_spmd(nc, in_maps, core_ids=list(range(N_CORES)))
    total = sum(float(r["out"][0, 0]) for r in res.results)
    return np.float32(total)



# revision 7
# speedup vs baseline: 1.9829x; 1.9829x over previous
"""Trainium2 Bass kernel for CRF negative-log-likelihood loss (v2).

Problem: nn_CRF (B=512, L=1024, T=48), data-parallel over 8 NeuronCores
(64 batch rows per core); host sums the 8 partial losses.

v2 design (vs the v1 unidirectional fp32 scan):
  - Bidirectional exact scan: forward chain (t=0..511) and backward
    adjoint chain (t=1023..512) run concurrently, stacked on partitions
    0-47 / 64-111 (gap 48-63 zeroed; PE/ACT partition bases must be
    multiples of 32). Each wall-step is ONE bf16 96x96-in-112x112 matmul
    (block-diag [E ; E^T] stationary) plus ONE 112-partition DVE
    Hadamard with exp(feat - MU) tiles. 512 serial steps instead of 1024.
  - Variable lengths: lengths >= L/2 guarantees the junction at t=511 is
    live for every row. The backward chain starts from zero state and a
    per-step rank-1 matmul injects exp(end) at t = len_b (indicator rows
    streamed from DRAM), so dead region stays exactly 0 and no per-step
    mask blending is needed. Z_b = alpha_511 . beta_511 at the junction.
  - No renormalization: drift of log-state over 512 steps is ~ +-15,
    far inside fp32/bf16 exponent range (validated in numpy).
  - Gold trans/start/end terms via ONE gpsimd ap_gather over a
    host-packed index tensor (masked entries hit a zero table slot),
    then ones-matmul reduction.
  - Gold feat term: per-(b,chunk) fused is_equal/mult/accumulate DVE ops
    against re-read t-major feat tiles, interleaved into scan bubbles.
  - All bulk DMAs issued from the gpsimd queue (25ns/issue vs 565 on SP).
"""

import math

import numpy as np
import ml_dtypes

import concourse.bacc as bacc
import concourse.mybir as mybir
import concourse.tile as tile
from concourse.bass_utils import run_bass_kernel_spmd

F32 = mybir.dt.float32
F32R = mybir.dt.float32r
BF16 = mybir.dt.bfloat16
I16 = mybir.dt.int16
I32 = mybir.dt.int32
AF = mybir.ActivationFunctionType
OP = mybir.AluOpType

B_FULL = 512
N_CORES = 8
BC = B_FULL // N_CORES          # 64
L_FULL = 1024
T = 48
MID = L_FULL // 2               # 512 junction
MU = 0.51
ASH = math.log(T)

FCH = 32                        # timesteps per natf chunk DMA
WIN = 8                         # steps per F tile window
NWIN = MID // WIN               # 64
NCH = MID // FCH                # 16 chunks per direction

NIDX = 1025                     # gather idxs per b: 1023 trans + start + end
NIDX_CORE = 8208                # 8*1025 rounded up to %16==0 (pad 2304)
TBL = 2401                      # 2304 trans + zero + 48 start + 48 end


def build_program(dbg=False):
    nc = bacc.Bacc("TRN2", target_bir_lowering=False, debug=False)

    feats_d = nc.dram_tensor("feats", (BC, L_FULL, T), F32,
                             kind="ExternalInput")
    wmain_d = nc.dram_tensor("wmain", (112, 112), BF16, kind="ExternalInput")
    werow_d = nc.dram_tensor("werow", (1, 112), BF16, kind="ExternalInput")
    wsrow_d = nc.dram_tensor("wsrow", (1, 112), BF16, kind="ExternalInput")
    ind_d = nc.dram_tensor("ind", (1, 65 * 512), BF16,
                           kind="ExternalInput")
    tagm_d = nc.dram_tensor("tagm", (128, 512), F32, kind="ExternalInput")
    idx_d = nc.dram_tensor("idxw", (128, NIDX_CORE // 16), I16,
                           kind="ExternalInput")
    tbl_d = nc.dram_tensor("tbl", (1, TBL), F32, kind="ExternalInput")
    shifts_d = nc.dram_tensor("shifts", (1, BC), F32, kind="ExternalInput")
    colsel_d = nc.dram_tensor("colsel", (128, 1), F32, kind="ExternalInput")
    out_d = nc.dram_tensor("out", (1, 1), F32, kind="ExternalOutput")
    dbg_d = (nc.dram_tensor("dbg", (4, BC), F32, kind="ExternalOutput")
             if dbg else None)

    feats_flat = feats_d.ap().rearrange("b l t -> b (l t)")

    with tile.TileContext(nc) as tc:
        with (
            tc.tile_pool(name="const", bufs=1) as cp,
            tc.tile_pool(name="natfp", bufs=3) as natp,
            tc.tile_pool(name="natbp", bufs=3) as natbp,
            tc.tile_pool(name="fgp", bufs=3) as fgp,
            tc.tile_pool(name="indp", bufs=3) as indwp,
            tc.tile_pool(name="ap", bufs=3) as apool,
            tc.tile_pool(name="scrp", bufs=2) as scrp,
            tc.tile_pool(name="tpfps", bufs=2, space="PSUM") as tpfp,
            tc.tile_pool(name="tpbps", bufs=2, space="PSUM") as tpbp,
            tc.tile_pool(name="scanps", bufs=2, space="PSUM") as scanp,
            tc.tile_pool(name="gps", bufs=1, space="PSUM") as gpsp,
        ):
            # ---------------- constants / params ----------------
            iota64i = cp.tile((64, 64), I32)
            nc.gpsimd.iota(iota64i[:, :], [[1, 64]], channel_multiplier=0)
            iota64f = cp.tile((64, 64), F32)
            nc.vector.tensor_copy(iota64f[:, :], iota64i[:, :])
            iotaPi = cp.tile((64, 1), I32)
            nc.gpsimd.iota(iotaPi[:, :], [[1, 1]], channel_multiplier=1)
            iotaPf = cp.tile((64, 1), F32)
            nc.vector.tensor_copy(iotaPf[:, :], iotaPi[:, :])
            identM = cp.tile((64, 64), F32)
            nc.vector.tensor_scalar(identM[:, :], iota64f[:, :],
                                    iotaPf[:, :], None, OP.is_equal)

            iota48i = cp.tile((128, T), I32)
            nc.gpsimd.iota(iota48i[:, :], [[1, T]], channel_multiplier=0)
            iota48f = cp.tile((128, T), F32)
            nc.vector.tensor_copy(iota48f[:, :], iota48i[:, :])

            ones64b = cp.tile((1, BC), BF16)
            nc.vector.memset(ones64b[:, :], 1.0)
            ones48c = cp.tile((T, 1), F32)
            nc.vector.memset(ones48c[:, :], 1.0)
            ones128c = cp.tile((128, 1), F32)
            nc.vector.memset(ones128c[:, :], 1.0)
            colsel = cp.tile((128, 1), F32)   # 1 at partitions p%16==0
            nc.gpsimd.dma_start(colsel[:, :], colsel_d.ap())

            bias_mu = cp.tile((T, 1), F32)
            nc.vector.memset(bias_mu[:, :], -MU)

            wmain = cp.tile((112, 112), BF16)
            nc.gpsimd.dma_start(wmain[:, :], wmain_d.ap())
            werow = cp.tile((1, 112), BF16)
            nc.gpsimd.dma_start(werow[:, :], werow_d.ap())
            wsrow = cp.tile((1, 112), BF16)
            nc.gpsimd.dma_start(wsrow[:, :], wsrow_d.ap())
            tagm = cp.tile((128, 512), F32)
            nc.gpsimd.dma_start(tagm[:, :], tagm_d.ap())
            idxw = cp.tile((128, NIDX_CORE // 16), I16)
            nc.gpsimd.dma_start(idxw[:, :], idx_d.ap())
            tbl = cp.tile((128, TBL), F32)
            nc.gpsimd.dma_start(tbl[:, :], tbl_d.ap().partition_broadcast(128))
            shifts = cp.tile((1, BC), F32)
            nc.gpsimd.dma_start(shifts[:, :], shifts_d.ap())

            # F tiles: 3 persistent buffers, gap rows zeroed once
            fbufs = []
            for i in range(3):
                fb = cp.tile((112, 512), BF16, name=f"fbuf{i}")
                nc.vector.memset(fb[:, :], 0.0)
                fbufs.append(fb)

            gout = cp.tile((128, NIDX_CORE), F32)
            feat_acc = cp.tile((128, 512), F32)

            gsum_ps = gpsp.tile((1, 512), F32, name="gsum")
            end_ps = gpsp.tile((1, 512), F32, name="endt")

            # ---------------- helper emitters ----------------
            natf_tiles = {}
            natb_tiles = {}

            def emit_chunk(c, bwd):
                pool = natbp if bwd else natp
                tl = pool.tile((BC, FCH * T), F32, name="natb" if bwd else
                               "natf")
                if bwd:
                    lo = (L_FULL - FCH * (c + 1)) * T
                else:
                    lo = FCH * c * T
                nc.gpsimd.dma_start(tl[:, :], feats_flat[:, lo:lo + FCH * T])
                (natb_tiles if bwd else natf_tiles)[c] = tl

            def emit_fprep(m):
                """F window m: steps 8m..8m+7 (fwd t=k, bwd t=1023-k)."""
                cf = m // 4
                tpf = tpfp.tile((T, 512), F32, name="tpf")
                tpb = tpbp.tile((T, 512), F32, name="tpb")
                tpfr = tpf
                tpbr = tpb
                nf = natf_tiles[cf]
                nb = natb_tiles[cf]
                for q in range(WIN):
                    col = (8 * m + q) % FCH            # fwd col in chunk
                    colb = 31 - 8 * (m % 4) - q        # bwd col in chunk
                    nc.tensor.matmul(
                        tpfr[:, 64 * q:64 * q + BC],
                        nf[:, T * col:T * (col + 1)],
                        identM[:, :], is_transpose=True, start=True,
                        stop=True, skip_group_check=True)
                    nc.tensor.matmul(
                        tpbr[:, 64 * q:64 * q + BC],
                        nb[:, T * colb:T * (colb + 1)],
                        identM[:, :], is_transpose=True, start=True,
                        stop=True, skip_group_check=True)
                fb = fbufs[m % 3]
                nc.scalar.activation(fb[0:T, :], tpf[:, :], AF.Exp,
                                     bias=bias_mu[:, :])
                nc.scalar.activation(fb[64:64 + T, :], tpb[:, :], AF.Exp,
                                     bias=bias_mu[:, :])

            ind_tiles = {}

            def emit_ind(m):
                """Indicator rows for steps 8m..8m+7 (+ step 512 in m=64)."""
                it = indwp.tile((1, 512), BF16, name="indw")
                nc.gpsimd.dma_start(
                    it[:, :], ind_d.ap()[:, 512 * m:512 * (m + 1)])
                ind_tiles[m] = it

            fg_tiles = {}

            def emit_fg(b):
                fgb = fgp.tile((128, 8 * T), F32, name="fg")
                nc.gpsimd.dma_start(
                    fgb[:, :].rearrange("p (c t) -> p c t", c=8, t=T),
                    feats_flat[b:b + 1, :].rearrange(
                        "o (c p t) -> (o p) c t", c=8, p=128, t=T))
                fg_tiles[b] = fgb

            def emit_stt(b, c):
                scr = scrp.tile((128, T), F32, name="scr")
                nc.vector.scalar_tensor_tensor(
                    scr[:, :], iota48f[:, :],
                    tagm[:, 8 * b + c:8 * b + c + 1],
                    fg_tiles[b][:, T * c:T * (c + 1)],
                    OP.is_equal, OP.mult,
                    accum_out=feat_acc[:, 8 * b + c:8 * b + c + 1])

            # ---------------- pipeline ----------------
            emit_chunk(0, False)
            emit_chunk(0, True)
            emit_chunk(1, False)
            emit_chunk(1, True)
            emit_ind(0)
            emit_ind(1)
            emit_fprep(0)

            a_prev = None
            jn = None
            gold_emitted = 0

            for k in range(MID + 1):
                m = k // WIN
                q = k % WIN
                if q == 0 and k < MID:
                    # prefetch: chunk m//4+2, ind window m+1, F window m+1
                    cnext = m // 4 + 2
                    if q == 0 and m % 4 == 0 and cnext < NCH:
                        emit_chunk(cnext, False)
                        emit_chunk(cnext, True)
                    if m + 2 <= NWIN:
                        emit_ind(m + 2)
                    if m + 1 < NWIN:
                        emit_fprep(m + 1)

                ps = scanp.tile((112, BC), F32, name="ps")
                if k == 0:
                    nc.tensor.matmul(ps[:, :], wsrow[:, :], ones64b[:, :],
                                     start=True, stop=False,
                                     skip_group_check=True)
                else:
                    nc.tensor.matmul(ps[:, :], wmain[:, :], a_prev[:, :],
                                     start=True, stop=False,
                                     skip_group_check=True)
                it = ind_tiles[m]
                nc.tensor.matmul(ps[:, :], werow[:, :],
                                 it[:, 64 * q:64 * (q + 1)],
                                 start=False, stop=True,
                                 skip_group_check=True)

                if k < MID:
                    fb = fbufs[m % 3]
                    a_cur = apool.tile((112, BC), BF16, name="a_t")
                    nc.vector.tensor_tensor(
                        a_cur[:, :], ps[:, :], fb[:, 64 * q:64 * (q + 1)],
                        OP.mult)
                    a_prev = a_cur
                else:
                    jn = cp.tile((T, BC), F32)
                    nc.vector.tensor_tensor(jn[:, :], ps[64:64 + T, :],
                                            a_prev[0:T, :], OP.mult)

                # interleaved gold-feat work: one stt per step
                if k == 1:
                    emit_fg(0)
                    emit_fg(1)
                if k >= 2 and gold_emitted < 512:
                    b, c = divmod(gold_emitted, 8)
                    if c == 0 and b + 2 < BC:
                        emit_fg(b + 2)
                    emit_stt(b, c)
                    gold_emitted += 1

            while gold_emitted < 512:
                b, c = divmod(gold_emitted, 8)
                emit_stt(b, c)
                gold_emitted += 1

            # ---------------- gather (gpsimd, end of its queue) --------
            nc.gpsimd.ap_gather(gout[:, :], tbl[:, :], idxw[:, :],
                                channels=128, num_elems=TBL, d=1,
                                num_idxs=NIDX_CORE)
            for s in range(16):
                nc.tensor.matmul(gsum_ps[:, :], colsel[:, :],
                                 gout[:, 512 * s:512 * (s + 1)],
                                 start=(s == 0), stop=False,
                                 skip_group_check=True)
            nc.tensor.matmul(gsum_ps[:, 0:16], colsel[:, :],
                             gout[:, 8192:8208],
                             start=False, stop=True,
                             skip_group_check=True)

            # ---------------- end phase ----------------
            zps = end_ps[0:1, 0:BC]
            nc.tensor.matmul(zps, ones48c[:, :], jn[:, :],
                             start=True, stop=True, skip_group_check=True)
            lnz = cp.tile((1, BC), F32)
            nc.scalar.activation(lnz[:, :], zps, AF.Ln)
            fwdrow = cp.tile((1, BC), F32)
            nc.vector.tensor_tensor(fwdrow[:, :], lnz[:, :], shifts[:, :],
                                    OP.add)
            fwd_tot = cp.tile((1, 1), F32)
            nc.vector.tensor_reduce(fwd_tot[:, :], fwdrow[:, :],
                                    mybir.AxisListType.X, OP.add)

            gtr = cp.tile((1, 1), F32)
            nc.vector.tensor_reduce(gtr[:, :], gsum_ps[:, :],
                                    mybir.AxisListType.X, OP.add)
            fred = cp.tile((128, 1), F32)
            nc.vector.tensor_reduce(fred[:, :], feat_acc[:, :],
                                    mybir.AxisListType.X, OP.add)
            gfe_ps = end_ps[0:1, 256:257]
            nc.tensor.matmul(gfe_ps, fred[:, :], ones128c[:, :],
                             start=True, stop=True, skip_group_check=True)
            loss = cp.tile((1, 1), F32)
            nc.vector.tensor_tensor(loss[:, :], fwd_tot[:, :], gtr[:, :],
                                    OP.subtract)
            nc.vector.tensor_tensor(loss[:, :], loss[:, :], gfe_ps,
                                    OP.subtract)
            nc.sync.dma_start(out_d.ap(), loss[:, :])
            if dbg:
                nc.sync.dma_start(dbg_d.ap()[0:1, :], lnz[:, :])
                nc.sync.dma_start(dbg_d.ap()[1:2, :], fwdrow[:, :])
                dbg2 = cp.tile((1, BC), F32)
                nc.scalar.copy(dbg2[:, :], gsum_ps[:, 0:BC])
                nc.sync.dma_start(dbg_d.ap()[2:3, :], dbg2[:, :])
                dbg3 = cp.tile((1, BC), F32)
                nc.scalar.copy(dbg3[:, :], jn[0:1, :])
                nc.sync.dma_start(dbg_d.ap()[3:4, :], dbg3[:, :])

    nc.compile()
    return nc


def shard_inputs(feats, transitions, start_transitions, end_transitions,
                 tags, mask, n_cores=N_CORES):
    feats = np.ascontiguousarray(np.asarray(feats, dtype=np.float32))
    trans = np.asarray(transitions, dtype=np.float32)
    start = np.asarray(start_transitions, dtype=np.float32)
    end = np.asarray(end_transitions, dtype=np.float32)
    tags = np.asarray(tags).astype(np.int64)
    mask = np.asarray(mask).astype(np.int64)
    B, L = tags.shape
    lens = mask.sum(1).astype(np.int64)              # (B,)

    E = np.exp(trans - ASH)
    wmain = np.zeros((112, 112), dtype=np.float32)
    wmain[0:T, 0:T] = E                               # fwd: out_j = sum_i E[i,j] a_i
    wmain[64:64 + T, 64:64 + T] = E.T                 # bwd: out_i = sum_j E[i,j] g_j
    werow = np.zeros((1, 112), dtype=np.float32)
    werow[0, 64:64 + T] = np.exp(end)
    wsrow = np.zeros((1, 112), dtype=np.float32)
    wsrow[0, 0:T] = np.exp(start)
    wmain16 = wmain.astype(ml_dtypes.bfloat16)
    werow16 = werow.astype(ml_dtypes.bfloat16)
    wsrow16 = wsrow.astype(ml_dtypes.bfloat16)

    tbl = np.zeros((1, TBL), dtype=np.float32)
    tbl[0, 0:2304] = trans.reshape(-1)
    tbl[0, 2305:2305 + T] = start
    tbl[0, 2353:2353 + T] = end

    tagm_full = np.where(mask > 0, tags, 300).astype(np.float32)  # (B, L)

    in_maps = []
    for cidx in range(n_cores):
        sl = slice(cidx * BC, (cidx + 1) * BC)
        f_c = feats[sl]
        tg = tags[sl]
        mk = mask[sl]
        ln = lens[sl]

        ind = np.zeros((65 * 512 // BC, BC), dtype=np.float32)
        ks = np.arange(MID + 1)
        ind[0:MID + 1, :] = (ln[None, :] == (L - ks)[:, None])
        ind = ind.reshape(1, -1).astype(ml_dtypes.bfloat16)

        tm = tagm_full[sl]                            # (BC, L)
        # tagm[p, 8b+c] = tm[b, 128c+p]
        tagm = np.ascontiguousarray(
            tm.reshape(BC, 8, 128).transpose(2, 0, 1).reshape(128, 512),
            dtype=np.float32)

        # gather idxs per core-half... 8 cores of gpsimd each own 8 b rows
        idxw = np.full((128, NIDX_CORE // 16), 2304, dtype=np.int16)
        for g in range(8):
            lst = []
            for bb in range(8):
                b = 8 * g + bb
                v = (tg[b, :-1] * T + tg[b, 1:]).astype(np.int64)
                v = np.where(mk[b, 1:] > 0, v, 2304)
                lst.append(v)
                lst.append([2305 + tg[b, 0],
                            2353 + tg[b, ln[b] - 1]])
            flat = np.concatenate([np.asarray(x, dtype=np.int64)
                                   for x in lst])
            assert flat.size == 8 * NIDX
            k = np.arange(flat.size)
            idxw[16 * g + (k % 16), k // 16] = flat
        shifts = ((ln - 1) * ASH + ln * MU).astype(np.float32)[None, :]
        colsel = ((np.arange(128) % 16) == 0).astype(np.float32)[:, None]

        in_maps.append({
            "feats": f_c,
            "wmain": wmain16,
            "werow": werow16,
            "wsrow": wsrow16,
            "ind": ind,
            "tagm": tagm,
            "idxw": idxw,
            "tbl": tbl,
            "shifts": shifts,
            "colsel": colsel,
        })
    return in_maps


def kernel(feats, transitions, start_transitions, end_transitions, tags,
           mask, **_ignored):
    in_maps = shard_inputs(feats, transitions, start_transitions,
                           end_transitions, tags, mask)
    nc = build_program()
    res = run_bass_kernel_spmd(nc, in_maps, core_ids=list(range(N_CORES)))
    total = sum(float(r["out"][0, 0]) for r in res.results)
    return np.float32(total)


# revision 13
# speedup vs baseline: 2.2801x; 1.1499x over previous
"""Trainium2 Bass kernel for CRF negative-log-likelihood loss (v4).

Problem: nn_CRF (B=512, L=1024, T=48), data-parallel over 8 NeuronCores
(64 batch rows per core); host sums the 8 partial losses.

Design:
  - Bidirectional exact scan: forward chain (t=0..511) and backward
    adjoint chain (t=1023..512) run concurrently, stacked on partitions
    0-47 / 64-111 of one (128, 64) state tile. 512 serial steps.
  - PE-quadrant-resident stationaries, loaded ONCE via standalone
    ldweights() and never reloaded (scan matmuls and transposes carry
    ldweights=False): quadrant (0,0) = fwd E block + exp(start) init
    row; quadrant (64,64) = bwd E^T block + exp(end) inject row;
    quadrant (64,0) = transpose identity. Each scan step is TWO 64-col
    quadrant matmuls (separate PSUM halves; cross-quadrant accumulation
    into one PSUM region crashes TRN2) plus ONE 128-partition DVE
    scalar_tensor_tensor: a_cur = max(ps, svec) * F.
  - The per-step indicator row ind_k (ind_k[b] = 1 iff len_b == 1024-k)
    rides at partition 112 of the state (a bwd-quadrant spare row whose
    stationary row is the exp(end) pattern). It is regenerated each
    step by the Hadamard: F tiles carry ind_{k+1} in row 112 (tiny DMA)
    and svec = e_112, so max(0, 1) * ind_{k+1} rewrites it while live
    rows (nonnegative) pass through max(ps, 0) untouched.
  - Variable lengths: lengths >= L/2 makes the junction at t=511 live
    for every row; backward dead region is exactly zero (no blending).
    Z_b = alpha_511 . beta_511, log-corrected by host-computed shifts.
  - No renormalization (drift ~ e^+-15 validated in numpy).
  - feats are converted to bf16 on host: halves HBM traffic, gives
    1-cycle/row PE transposes and 2x DVE mode for the gold-feat ops.
  - Gold trans/start/end terms via one gpsimd ap_gather over a
    host-packed index tensor; gold feat term via per-(b,chunk) fused
    is_equal/mult/accumulate DVE ops interleaved into scan bubbles.
"""

import math

import numpy as np
import ml_dtypes

import concourse.bacc as bacc
import concourse.mybir as mybir
import concourse.tile as tile
from concourse.bass_utils import run_bass_kernel_spmd

F32 = mybir.dt.float32
BF16 = mybir.dt.bfloat16
I16 = mybir.dt.int16
I32 = mybir.dt.int32
AF = mybir.ActivationFunctionType
OP = mybir.AluOpType

B_FULL = 512
N_CORES = 8
BC = B_FULL // N_CORES          # 64
L_FULL = 1024
T = 48
MID = L_FULL // 2               # 512 junction
MU = 0.51
ASH = math.log(T)

FCH = 32                        # timesteps per natf chunk DMA
WIN = 8                         # steps per F tile window
NWIN = MID // WIN               # 64
NCH = MID // FCH                # 16 chunks per direction

NIDX = 1025                     # gather idxs per b: 1023 trans + start + end
NIDX_CORE = 8208                # 8*1025 rounded up to %16==0 (pad 2304)
TBL = 2401                      # 2304 trans + zero + 48 start + 48 end


def build_program(dbg=False):
    nc = bacc.Bacc("TRN2", target_bir_lowering=False, debug=False)

    feats_d = nc.dram_tensor("feats", (BC, L_FULL, T), BF16,
                             kind="ExternalInput")
    wfwd_d = nc.dram_tensor("wfwd", (64, 64), BF16, kind="ExternalInput")
    wbwd_d = nc.dram_tensor("wbwd", (64, 64), BF16, kind="ExternalInput")
    ident_d = nc.dram_tensor("ident", (64, 64), BF16, kind="ExternalInput")
    ainit_d = nc.dram_tensor("ainit", (128, BC), BF16, kind="ExternalInput")
    svec_d = nc.dram_tensor("svec", (128, 1), F32, kind="ExternalInput")
    indf_d = nc.dram_tensor("indf", (1, NWIN * 512), BF16,
                            kind="ExternalInput")
    tagm_d = nc.dram_tensor("tagm", (128, 512), F32, kind="ExternalInput")
    idx_d = nc.dram_tensor("idxw", (128, NIDX_CORE // 16), I16,
                           kind="ExternalInput")
    tbl_d = nc.dram_tensor("tbl", (1, TBL), F32, kind="ExternalInput")
    shifts_d = nc.dram_tensor("shifts", (1, BC), F32, kind="ExternalInput")
    colsel_d = nc.dram_tensor("colsel", (128, 1), F32, kind="ExternalInput")
    out_d = nc.dram_tensor("out", (1, 1), F32, kind="ExternalOutput")
    dbg_d = (nc.dram_tensor("dbg", (4, BC), F32, kind="ExternalOutput")
             if dbg else None)

    feats_flat = feats_d.ap().rearrange("b l t -> b (l t)")

    with tile.TileContext(nc) as tc:
        with (
            tc.tile_pool(name="const", bufs=1) as cp,
            tc.tile_pool(name="natfp", bufs=3) as natp,
            tc.tile_pool(name="natbp", bufs=3) as natbp,
            tc.tile_pool(name="fgp", bufs=3) as fgp,
            tc.tile_pool(name="ap", bufs=3) as apool,
            tc.tile_pool(name="scrp", bufs=2) as scrp,
            tc.tile_pool(name="tpfps", bufs=2, space="PSUM") as tpfp,
            tc.tile_pool(name="tpbps", bufs=2, space="PSUM") as tpbp,
            tc.tile_pool(name="scanps", bufs=2, space="PSUM") as scanp,
            tc.tile_pool(name="gps", bufs=1, space="PSUM") as gpsp,
        ):
            # ---------------- constants / params ----------------
            identM = cp.tile((128, 64), BF16)
            nc.sync.dma_start(identM[64:128, :], ident_d.ap())

            iota48i = cp.tile((128, T), I32)
            nc.gpsimd.iota(iota48i[:, :], [[1, T]], channel_multiplier=0)
            iota48b = cp.tile((128, T), BF16)
            nc.vector.tensor_copy(iota48b[:, :], iota48i[:, :])

            bias_mu = cp.tile((T, 1), F32)
            nc.vector.memset(bias_mu[:, :], -MU)
            ones48c = cp.tile((T, 1), F32)
            nc.vector.memset(ones48c[:, :], 1.0)
            ones128c = cp.tile((128, 1), F32)
            nc.vector.memset(ones128c[:, :], 1.0)
            colsel = cp.tile((128, 1), F32)
            nc.sync.dma_start(colsel[:, :], colsel_d.ap())
            svec = cp.tile((128, 1), F32)
            nc.sync.dma_start(svec[:, :], svec_d.ap())

            wfwd = cp.tile((64, 64), BF16)
            nc.sync.dma_start(wfwd[:, :], wfwd_d.ap())
            wbwd = cp.tile((128, 64), BF16)
            nc.sync.dma_start(wbwd[64:128, :], wbwd_d.ap())
            tagm = cp.tile((128, 512), F32)
            nc.sync.dma_start(tagm[:, :], tagm_d.ap())
            idxw = cp.tile((128, NIDX_CORE // 16), I16)
            nc.sync.dma_start(idxw[:, :], idx_d.ap())
            tbl = cp.tile((128, TBL), F32)
            nc.sync.dma_start(tbl[:, :], tbl_d.ap().partition_broadcast(128))
            shifts = cp.tile((1, BC), F32)
            nc.sync.dma_start(shifts[:, :], shifts_d.ap())

            # F tiles: 3 persistent buffers, gap rows zeroed once
            fbufs = []
            for i in range(3):
                fb = cp.tile((128, 512), BF16, name=f"fbuf{i}")
                nc.vector.memset(fb[:, :], 0.0)
                fbufs.append(fb)

            a_init = cp.tile((128, BC), BF16)
            nc.sync.dma_start(a_init[:, :], ainit_d.ap())

            nc.tensor.ldweights(wfwd[:, :], tile_position=(0, 0))
            nc.tensor.ldweights(wbwd[64:128, :], tile_position=(64, 64))
            nc.tensor.ldweights(identM[64:128, :], is_transpose=True,
                                tile_position=(64, 0))

            gout = cp.tile((128, NIDX_CORE), F32)
            feat_acc = cp.tile((128, 512), F32)

            gsum_ps = gpsp.tile((1, 512), F32, name="gsum")
            end_ps = gpsp.tile((1, 512), F32, name="endt")

            # ---------------- helper emitters ----------------
            natf_tiles = {}
            natb_tiles = {}

            def emit_chunk(c, bwd):
                pool = natbp if bwd else natp
                tl = pool.tile((128, FCH * T), BF16,
                               name="natb" if bwd else "natf")
                if bwd:
                    lo = (L_FULL - FCH * (c + 1)) * T
                else:
                    lo = FCH * c * T
                nc.gpsimd.dma_start(tl[64:128, :],
                                    feats_flat[:, lo:lo + FCH * T])
                (natb_tiles if bwd else natf_tiles)[c] = tl

            def emit_fprep(m):
                """F window m: steps 8m..8m+7 (fwd t=k, bwd t=1023-k)."""
                cf = m // 4
                tpf = tpfp.tile((T, 512), BF16, name="tpf")
                tpb = tpbp.tile((T, 512), BF16, name="tpb")
                nf = natf_tiles[cf]
                nb = natb_tiles[cf]
                for q in range(WIN):
                    colf = 8 * (m % 4) + q
                    colb = 31 - 8 * (m % 4) - q
                    r = nc.tensor.matmul(
                        tpf[:, 64 * q:64 * q + BC],
                        nf[64:128, T * colf:T * (colf + 1)],
                        identM[64:128, :], is_transpose=True, start=True,
                        stop=True, skip_group_check=True,
                        tile_position=(64, 0))
                    r.ins.ldweights = False
                    r = nc.tensor.matmul(
                        tpb[:, 64 * q:64 * q + BC],
                        nb[64:128, T * colb:T * (colb + 1)],
                        identM[64:128, :], is_transpose=True, start=True,
                        stop=True, skip_group_check=True,
                        tile_position=(64, 0))
                    r.ins.ldweights = False
                fb = fbufs[m % 3]
                nc.scalar.activation(fb[0:T, :], tpf[:, :], AF.Exp,
                                     bias=bias_mu[:, :])
                nc.scalar.activation(fb[64:64 + T, :], tpb[:, :], AF.Exp,
                                     bias=bias_mu[:, :])
                # indicator row for steps 8m+1 .. 8m+8
                nc.sync.dma_start(fb[112:113, :],
                                  indf_d.ap()[:, 512 * m:512 * (m + 1)])

            fg_tiles = {}

            def emit_fg(b):
                fgb = fgp.tile((128, 8 * T), BF16, name="fg")
                nc.gpsimd.dma_start(
                    fgb[:, :].rearrange("p (c t) -> p c t", c=8, t=T),
                    feats_flat[b:b + 1, :].rearrange(
                        "o (c p t) -> (o p) c t", c=8, p=128, t=T))
                fg_tiles[b] = fgb

            def emit_stt(b, c):
                scr = scrp.tile((128, T), BF16, name="scr")
                nc.vector.scalar_tensor_tensor(
                    scr[:, :], iota48b[:, :],
                    tagm[:, 8 * b + c:8 * b + c + 1],
                    fg_tiles[b][:, T * c:T * (c + 1)],
                    OP.is_equal, OP.mult,
                    accum_out=feat_acc[:, 8 * b + c:8 * b + c + 1])

            # ---------------- pipeline ----------------
            emit_chunk(0, False)
            emit_chunk(0, True)
            emit_chunk(1, False)
            emit_chunk(1, True)
            emit_fprep(0)

            a_prev = a_init
            jn = None
            gold_emitted = 0

            for k in range(MID + 1):
                m = k // WIN
                q = k % WIN
                if q == 0 and k < MID:
                    cnext = m // 4 + 2
                    if m % 4 == 0 and cnext < NCH:
                        emit_chunk(cnext, False)
                        emit_chunk(cnext, True)
                    if m + 1 < NWIN:
                        emit_fprep(m + 1)

                ps = scanp.tile((128, BC), F32, name="ps")
                r = nc.tensor.matmul(ps[0:64, :], wfwd[:, :],
                                     a_prev[0:64, :],
                                     start=True, stop=True,
                                     skip_group_check=True,
                                     tile_position=(0, 0))
                r.ins.ldweights = False
                r = nc.tensor.matmul(ps[64:128, :], wbwd[64:128, :],
                                     a_prev[64:128, :],
                                     start=True, stop=True,
                                     skip_group_check=True,
                                     tile_position=(64, 64))
                r.ins.ldweights = False

                if k < MID:
                    fb = fbufs[m % 3]
                    a_cur = apool.tile((128, BC), BF16, name="a_t")
                    nc.vector.scalar_tensor_tensor(
                        a_cur[:, :], ps[:, :], svec[:, :],
                        fb[:, 64 * q:64 * (q + 1)],
                        OP.max, OP.mult)
                    a_prev = a_cur
                else:
                    jn = cp.tile((T, BC), F32)
                    nc.vector.tensor_tensor(jn[:, :], ps[64:64 + T, :],
                                            a_prev[0:T, :], OP.mult)

                # interleaved gold-feat work: one stt per step
                if k == 1:
                    emit_fg(0)
                    emit_fg(1)
                if k >= 2 and gold_emitted < 512:
                    b, c = divmod(gold_emitted, 8)
                    if c == 0 and b + 2 < BC:
                        emit_fg(b + 2)
                    emit_stt(b, c)
                    gold_emitted += 1

            while gold_emitted < 512:
                b, c = divmod(gold_emitted, 8)
                emit_stt(b, c)
                gold_emitted += 1

            # ---------------- gather (gpsimd, end of its queue) --------
            nc.gpsimd.ap_gather(gout[:, :], tbl[:, :], idxw[:, :],
                                channels=128, num_elems=TBL, d=1,
                                num_idxs=NIDX_CORE)
            for s in range(16):
                nc.tensor.matmul(gsum_ps[:, :], colsel[:, :],
                                 gout[:, 512 * s:512 * (s + 1)],
                                 start=(s == 0), stop=False,
                                 skip_group_check=True)
            nc.tensor.matmul(gsum_ps[:, 0:16], colsel[:, :],
                             gout[:, 8192:8208],
                             start=False, stop=True,
                             skip_group_check=True)

            # ---------------- end phase ----------------
            zps = end_ps[0:1, 0:BC]
            nc.tensor.matmul(zps, ones48c[:, :], jn[:, :],
                             start=True, stop=True, skip_group_check=True)
            lnz = cp.tile((1, BC), F32)
            nc.scalar.activation(lnz[:, :], zps, AF.Ln)
            fwdrow = cp.tile((1, BC), F32)
            nc.vector.tensor_tensor(fwdrow[:, :], lnz[:, :], shifts[:, :],
                                    OP.add)
            fwd_tot = cp.tile((1, 1), F32)
            nc.vector.tensor_reduce(fwd_tot[:, :], fwdrow[:, :],
                                    mybir.AxisListType.X, OP.add)

            gtr = cp.tile((1, 1), F32)
            nc.vector.tensor_reduce(gtr[:, :], gsum_ps[:, :],
                                    mybir.AxisListType.X, OP.add)
            fred = cp.tile((128, 1), F32)
            nc.vector.tensor_reduce(fred[:, :], feat_acc[:, :],
                                    mybir.AxisListType.X, OP.add)
            gfe_ps = end_ps[0:1, 256:257]
            nc.tensor.matmul(gfe_ps, fred[:, :], ones128c[:, :],
                             start=True, stop=True, skip_group_check=True)
            loss = cp.tile((1, 1), F32)
            nc.vector.tensor_tensor(loss[:, :], fwd_tot[:, :], gtr[:, :],
                                    OP.subtract)
            nc.vector.tensor_tensor(loss[:, :], loss[:, :], gfe_ps,
                                    OP.subtract)
            nc.sync.dma_start(out_d.ap(), loss[:, :])
            if dbg:
                nc.sync.dma_start(dbg_d.ap()[0:1, :], lnz[:, :])
                nc.sync.dma_start(dbg_d.ap()[1:2, :], fwdrow[:, :])
                dbg2 = cp.tile((1, BC), F32)
                nc.scalar.copy(dbg2[:, :], gsum_ps[:, 0:BC])
                nc.sync.dma_start(dbg_d.ap()[2:3, :], dbg2[:, :])
                dbg3 = cp.tile((1, BC), F32)
                nc.scalar.copy(dbg3[:, :], jn[0:1, :])
                nc.sync.dma_start(dbg_d.ap()[3:4, :], dbg3[:, :])

    nc.compile()
    return nc


def shard_inputs(feats, transitions, start_transitions, end_transitions,
                 tags, mask, n_cores=N_CORES):
    feats = np.asarray(feats, dtype=np.float32)
    trans = np.asarray(transitions, dtype=np.float32)
    start = np.asarray(start_transitions, dtype=np.float32)
    end = np.asarray(end_transitions, dtype=np.float32)
    tags = np.asarray(tags).astype(np.int64)
    mask = np.asarray(mask).astype(np.int64)
    B, L = tags.shape
    lens = mask.sum(1).astype(np.int64)              # (B,)
    feats16 = np.ascontiguousarray(feats.astype(ml_dtypes.bfloat16))

    E = np.exp(trans - ASH)
    wfwd = np.zeros((64, 64), dtype=np.float32)
    wfwd[0:T, 0:T] = E                    # fwd: out_j = sum_i E[i,j] a_i
    wfwd[49, 0:T] = np.exp(start)         # fwd init via ones row
    wbwd = np.zeros((64, 64), dtype=np.float32)
    wbwd[0:T, 0:T] = E.T                  # bwd: out_i = sum_j E[i,j] g_j
    wbwd[48, 0:T] = np.exp(end)           # inject row (partition 112)
    wfwd16 = wfwd.astype(ml_dtypes.bfloat16)
    wbwd16 = wbwd.astype(ml_dtypes.bfloat16)
    ident16 = np.eye(64, dtype=np.float32).astype(ml_dtypes.bfloat16)

    svec = np.zeros((128, 1), dtype=np.float32)
    svec[112, 0] = 1.0

    tbl = np.zeros((1, TBL), dtype=np.float32)
    tbl[0, 0:2304] = trans.reshape(-1)
    tbl[0, 2305:2305 + T] = start
    tbl[0, 2353:2353 + T] = end

    tagm_full = np.where(mask > 0, tags, 300).astype(np.float32)  # (B, L)

    in_maps = []
    for cidx in range(n_cores):
        sl = slice(cidx * BC, (cidx + 1) * BC)
        tg = tags[sl]
        mk = mask[sl]
        ln = lens[sl]

        # F-tile indicator rows: window m col 64q+b = (len_b == 1023-8m-q)
        ks = np.arange(1, MID + 1)                    # ind_{k+1} for k=8m+q
        indf = (ln[None, :] == (L - ks)[:, None]).astype(np.float32)
        indf = np.ascontiguousarray(
            indf.reshape(1, -1)).astype(ml_dtypes.bfloat16)

        ainit = np.zeros((128, BC), dtype=np.float32)
        ainit[112, :] = (ln == L)                     # ind_0
        ainit[49, :] = 1.0                            # fwd init ones row
        ainit16 = ainit.astype(ml_dtypes.bfloat16)

        tm = tagm_full[sl]                            # (BC, L)
        tagm = np.ascontiguousarray(
            tm.reshape(BC, 8, 128).transpose(2, 0, 1).reshape(128, 512),
            dtype=np.float32)

        idxw = np.full((128, NIDX_CORE // 16), 2304, dtype=np.int16)
        for g in range(8):
            lst = []
            for bb in range(8):
                b = 8 * g + bb
                v = (tg[b, :-1] * T + tg[b, 1:]).astype(np.int64)
                v = np.where(mk[b, 1:] > 0, v, 2304)
                lst.append(v)
                lst.append([2305 + tg[b, 0],
                            2353 + tg[b, ln[b] - 1]])
            flat = np.concatenate([np.asarray(x, dtype=np.int64)
                                   for x in lst])
            assert flat.size == 8 * NIDX
            kk = np.arange(flat.size)
            idxw[16 * g + (kk % 16), kk // 16] = flat
        shifts = ((ln - 1) * ASH + ln * MU).astype(np.float32)[None, :]
        colsel = ((np.arange(128) % 16) == 0).astype(np.float32)[:, None]

        in_maps.append({
            "feats": feats16[sl],
            "wfwd": wfwd16,
            "wbwd": wbwd16,
            "ident": ident16,
            "ainit": ainit16,
            "svec": svec,
            "indf": indf,
            "tagm": tagm,
            "idxw": idxw,
            "tbl": tbl,
            "shifts": shifts,
            "colsel": colsel,
        })
    return in_maps


def kernel(feats, transitions, start_transitions, end_transitions, tags,
           mask, **_ignored):
    in_maps = shard_inputs(feats, transitions, start_transitions,
                           end_transitions, tags, mask)
    nc = build_program()
    res = run_bass_kernel_spmd(nc, in_maps, core_ids=list(range(N_CORES)))
    total = sum(float(r["out"][0, 0]) for r in res.results)
    return np.float32(total)
